# revision 7
# baseline (speedup 1.0000x reference)
"""Bidirectional GINConv on 8 Trainium2 NeuronCores.

Strategy (dst-node sharding, zero collectives):
  - Pad node space to 50176 = 8 * 49 * 128; core k owns the 49 dst tiles
    (128 nodes each) of range [k*6272, (k+1)*6272).
  - Host groups edges (plus one synthetic self-edge per node, implementing
    the `x + agg` term) by (dst tile, src half, direction) and pads each
    bucket to a per-slot chunk count (max over the 8 cores, so the SPMD
    program is shape-uniform); src indices become int16-local offsets.
  - Device: for each (tile, half) one `dma_gather` fetches x rows (encoded
    as bf16 hi|lo pairs, 512B/row) for BOTH directions; a DVE is_equal
    against an iota builds one-hot dst matrices; the PE accumulates
    agg = sum(onehot^T @ x) into PSUM (hi and lo column blocks side by
    side -> exact-ish fp32 after one add).
  - agg(hi)+agg(lo) -> h, PE-transpose, then the 2-layer MLP per direction,
    directions summed in PSUM, final relu((a+b)/2 + b2) on ACT, store.
  - Host concatenates the per-core [128, 6272] outputs and transposes.
"""

import sys

import numpy as np
import ml_dtypes

sys.path.insert(0, "/opt/trn_rl_repo")

P = 128
D = 128
N_NODES = 50000
N_EDGES = 800000
N_CORES = 8
TILES_PER_CORE = 49
NODES_PER_CORE = TILES_PER_CORE * P      # 6272
TABLE_ROWS = N_CORES * NODES_PER_CORE    # 50176
HALF = TABLE_ROWS // 2                   # 25088

_BF16 = ml_dtypes.bfloat16
assert True  # ROW_MODE "hilo" no longer supported (aggT operand swap)
SINGLE_PACKET = False  # True crashes the runtime (verified)
# "f16": x rows stored as single fp16 (256B gathers, ~3e-4 rel err)
# "hilo": x rows stored as bf16 hi|lo pairs (512B gathers, ~2e-6 rel err)
ROW_MODE = "f16"
# dev-only ablations for benching: subset of {"no_gather", "no_compute"}
ABLATE = set()
# "dma_gather": int16 gather, x table split in halves (2 groups)
# "indirect": indirect_dma_start with int32 offsets, single group
GATHER_MODE = "dma_gather"
# sort each bucket's edges by src id -> ascending HBM addresses per DMA
# engine stream (row-buffer locality)
SORT_SRC = True
# dma_gather ucode runs each op on ONE Q7 core pair picked by queue_num
# (cpu_id/2 == queue_num); round-robin over 4 queues -> 4x descriptor
# emission parallelism (8 cores instead of 2)
N_SWDGE_QUEUES = 4
# USE_QUEUES must divide the 8 DMASW sem lanes: lane k's consecutive
# gathers must stay on one queue (per-queue FIFO completion) or false
# cross-queue waits serialize the pipeline (3 queues measured WORSE).
USE_QUEUES = 4
# pad tail of dir-1 idx region with -1 so the Q7 trims it at runtime.
# Trim only at whole-128-chunk granularity: a non-multiple-of-128 trimmed
# count triggers the ucode's partial-chunk dummy-descriptor path, which
# crashes this runtime (verified empirically).
TRIM_PAD = False


def _n_groups():
    return 2 if GATHER_MODE == "dma_gather" else 1


def _bucketize(src, dst, n_tiles_total, ng):
    """Group (src, dst) edges by (dst tile[, src half])."""
    s = np.asarray(src, np.int64)
    t = np.asarray(dst, np.int64)
    tile_id = t >> 7
    grp = (s >= HALF).astype(np.int64) if ng == 2 else np.zeros_like(s)
    key = tile_id * ng + grp
    if SORT_SRC:
        # secondary sort by src id within each bucket
        order = np.lexsort((s, key))
    else:
        order = np.argsort(key, kind="stable")
    s_s = s[order]
    dl_s = (t[order] & 127)
    key_s = key[order]
    counts = np.bincount(key_s, minlength=n_tiles_total * ng)
    return s_s, dl_s, key_s, counts


def _host_prep(x, edge_index, reverse_edge_index):
    """Build per-core device input arrays.

    Returns (ch_slot, idx_cores, dstv_cores, xt, iota) where
    ch_slot[tl][g][d] = chunk count for that slot (uniform across cores).
    """
    ng = _n_groups()
    icu = 8 if GATHER_MODE == "dma_gather" else 1  # idx cols per chunk
    n_tiles_total = N_CORES * TILES_PER_CORE  # 392

    packed = []
    for ei in (edge_index, reverse_edge_index):
        packed.append(_bucketize(ei[0], ei[1], n_tiles_total, ng))

    # per-slot chunk counts: max over cores
    ch_slot = np.zeros((TILES_PER_CORE, ng, 2), np.int64)  # [tl, g, d]
    for d, (_, _, _, counts) in enumerate(packed):
        c = counts.reshape(N_CORES, TILES_PER_CORE, ng)  # [core, tl, grp]
        ch_slot[:, :, d] = np.maximum(
            ch_slot[:, :, d], -(-c.max(axis=0) // P))

    toti = int(ch_slot.sum()) * icu  # idx cols per core
    totd = int(ch_slot.sum())        # dstv chunk-cols per core
    # column offsets per (tile, group, dir)
    idx_off = np.zeros((TILES_PER_CORE, ng, 2), np.int64)
    dstv_off = np.zeros((TILES_PER_CORE, ng, 2), np.int64)
    acc = 0
    for tl in range(TILES_PER_CORE):
        for g in range(ng):
            for d in (0, 1):
                idx_off[tl, g, d] = acc * icu
                dstv_off[tl, g, d] = acc
                acc += int(ch_slot[tl, g, d])

    mdt_np = np.float16 if ROW_MODE == "f16" else _BF16
    idt_np = np.int16 if GATHER_MODE == "dma_gather" else np.int32
    idx_cores = np.zeros((N_CORES, P, toti), idt_np)
    dstv_cores = np.full((N_CORES, P, 2 * totd), -1.0, mdt_np)

    for d, (s_s, dl_s, key_s, counts) in enumerate(packed):
        offs = np.zeros(n_tiles_total * ng + 1, dtype=np.int64)
        np.cumsum(counts, out=offs[1:])
        for b in range(n_tiles_total * ng):
            n = int(counts[b])
            if n == 0:
                continue
            tile, g = divmod(b, ng)
            core, tl = divmod(tile, TILES_PER_CORE)
            cap = int(ch_slot[tl, g, d]) * P
            o = offs[b]
            io = idx_off[tl, g, d]
            if GATHER_MODE == "dma_gather":
                src_l = np.zeros(cap, np.int16)
                src_l[:n] = (s_s[o:o + n] - g * HALF).astype(np.int16)
                if d == 1 and TRIM_PAD:
                    # dir-1 region is the tail of the (tile, half) gather;
                    # trailing -1 idxs are trimmed by the Q7 at runtime
                    # (per-core actual counts, not the 8-core max). Keep the
                    # partial chunk 0-padded; only whole empty chunks get -1.
                    n128 = -(-n // P) * P
                    src_l[n128:] = -1
                # slot i -> [i % 16, i // 16], replicated to 8 groups of 16
                iw = src_l.reshape(cap // 16, 16).T  # [16, cap//16]
                idx_cores[core, :, io:io + cap // 16] = np.tile(iw, (8, 1))
            else:
                src_l = np.zeros(cap, np.int32)
                src_l[:n] = s_s[o:o + n].astype(np.int32)
                # slot i -> [i % 128, i // 128]
                idx_cores[core, :, io:io + cap // P] = \
                    src_l.reshape(cap // P, P).T
            dl = np.full(cap, -1.0, mdt_np)
            dl[:n] = dl_s[o:o + n].astype(np.float32).astype(mdt_np)
            # dstv: slot i -> [i % 128, i // 128], each value duplicated into
            # an adjacent pair so the one-hot is_equal reads it with a
            # unit-stride 2-element last dim (DVE 2x_1P mode)
            do = dstv_off[tl, g, d]
            dw = dl.reshape(cap // P, P).T  # [128, chunks]
            dstv_cores[core, :, 2 * do:2 * (do + cap // P)] = \
                np.repeat(dw, 2, axis=1)

    x = np.asarray(x, np.float32)
    if ROW_MODE == "f16":
        # x table: [TABLE_ROWS, 128] fp16 rows
        xt = np.zeros((TABLE_ROWS, D), np.float16)
        xt[:N_NODES] = x.astype(np.float16)
        mdt = np.float16
    else:
        # x table: [TABLE_ROWS, 256] bf16, row = [hi(128) | lo(128)]
        xt = np.zeros((TABLE_ROWS, 2 * D), _BF16)
        hi = x.astype(_BF16)
        lo = (x - hi.astype(np.float32)).astype(_BF16)
        xt[:N_NODES, :D] = hi
        xt[:N_NODES, D:] = lo
        mdt = _BF16

    ch_max = int(ch_slot.sum(axis=2).max())  # widest (tile, half) gather
    iota = np.tile(np.arange(P, dtype=np.float32),
                   (P, ch_max, 1)).astype(mdt)

    # exact f32 x, sharded by core, TRANSPOSED to [D, nodes] (adds the GIN
    # "+x" term without gather; transposed so agg can accumulate as
    # aggT[feat, dst] and the MLP needs no PE transpose)
    xf = np.zeros((TABLE_ROWS, D), np.float32)
    xf[:N_NODES] = x
    xf_cores = xf.reshape(N_CORES, NODES_PER_CORE, D)
    xft_cores = np.ascontiguousarray(xf_cores.transpose(0, 2, 1))
    return ch_slot, idx_off, dstv_off, toti, totd, idx_cores, dstv_cores, \
        xt, iota, ch_max, xft_cores


def _build_program(ch_slot, idx_off, dstv_off, toti, totd, ch_max,
                   n_tiles, enable_asserts=False, bench_reps=0):
    import contextlib
    from concourse import bacc, mybir
    import concourse.tile as tile
    from concourse.masks import make_identity

    dt = mybir.dt
    if ROW_MODE == "f16":
        mdt = dt.float16
        xw = D
    else:
        mdt = dt.bfloat16
        xw = 2 * D

    nc = bacc.Bacc(
        "TRN2",
        target_bir_lowering=False,
        debug=False,
        enable_asserts=enable_asserts,
        num_devices=1,
        # scratch/16/4queues = per-queue ring descs: 64KB -> 1024-desc rings
        # (~7 gathers deep) so Q7 emission never stalls in await_space
        dynamic_dma_scratch_size=65536,
        **({"num_swdge_queues": N_SWDGE_QUEUES} if N_SWDGE_QUEUES > 1 else {}),
    )

    idt = dt.int16 if GATHER_MODE == "dma_gather" else dt.int32
    xt = nc.dram_tensor(
        "xt", [TABLE_ROWS, xw], mdt, kind="ExternalInput").ap()
    idx = nc.dram_tensor(
        "idx", [P, toti], idt, kind="ExternalInput").ap()
    dstv = nc.dram_tensor(
        "dstv", [P, 2 * totd], mdt, kind="ExternalInput").ap()
    iotar = nc.dram_tensor(
        "iotar", [P, ch_max, P], mdt, kind="ExternalInput").ap()
    w1t = nc.dram_tensor(
        "w1t", [D, D], dt.float32, kind="ExternalInput").ap()
    w2t = nc.dram_tensor(
        "w2t", [D, D], dt.float32, kind="ExternalInput").ap()
    b1c = nc.dram_tensor(
        "b1c", [D, 1], dt.float32, kind="ExternalInput").ap()
    b2c = nc.dram_tensor(
        "b2c", [D, 1], dt.float32, kind="ExternalInput").ap()
    xf = nc.dram_tensor(
        "xf", [D, NODES_PER_CORE], dt.float32, kind="ExternalInput").ap()
    y = nc.dram_tensor(
        "y", [D, n_tiles * P], dt.float32, kind="ExternalOutput").ap()

    with tile.TileContext(nc) as tc:
        with (
            tc.tile_pool(name="const", bufs=1) as cpool,
            tc.tile_pool(name="xgp", bufs=10) as xgpool,
            tc.tile_pool(name="mp", bufs=10) as mpool,
            tc.tile_pool(name="fp", bufs=12) as fpool,
            tc.tile_pool(name="aggps", bufs=4, space="PSUM") as aggpool,
            tc.tile_pool(name="mlpps", bufs=4, space="PSUM") as mlppool,
        ):
            # idx/dstv resident in SBUF: two big line-rate DMAs replace ~200
            # per-tile small-descriptor loads (and their dependency chains).
            # idx_all FIRST: the first gather only needs it, so the gather
            # pipeline starts while the other constants still load.
            idx_all = cpool.tile([P, toti], idt)
            nc.sync.dma_start(out=idx_all[:], in_=idx[:])
            dstv_all = cpool.tile([P, 2 * totd], mdt)
            nc.sync.dma_start(out=dstv_all[:], in_=dstv[:])
            iota_sb = cpool.tile([P, ch_max, P], mdt)
            nc.sync.dma_start(out=iota_sb[:], in_=iotar[:])
            w1t_sb = cpool.tile([D, D], dt.float32)
            nc.sync.dma_start(out=w1t_sb[:], in_=w1t[:])
            w2t_sb = cpool.tile([D, D], dt.float32)
            nc.sync.dma_start(out=w2t_sb[:], in_=w2t[:])
            b1_sb = cpool.tile([D, 1], dt.float32)
            nc.sync.dma_start(out=b1_sb[:], in_=b1c[:])
            b2_sb = cpool.tile([D, 1], dt.float32)
            nc.sync.dma_start(out=b2_sb[:], in_=b2c[:])
            ident = cpool.tile([P, P], dt.float32)
            make_identity(nc, ident[:])

            loop_cm = (tc.For_i(0, bench_reps, 1) if bench_reps
                       else contextlib.nullcontext())
            with loop_cm:
                _build_tiles(
                    nc, tc, mybir, dt, mdt, idt, xw, n_tiles, ch_slot,
                    idx_off, dstv_off, idx_all, dstv_all, xf, y, xt, iota_sb,
                    w1t_sb, w2t_sb, b1_sb, b2_sb, ident, xgpool,
                    mpool, fpool, aggpool, mlppool)

    nc.compile()
    return nc


def _build_tiles(nc, tc, mybir, dt, mdt, idt, xw, n_tiles, ch_slot, idx_off,
                 dstv_off, idx_all, dstv_all, xf, y, xt, iota_sb, w1t_sb,
                 w2t_sb, b1_sb, b2_sb, ident, xgpool, mpool, fpool,
                 aggpool, mlppool):
    ng = _n_groups()
    # Greedy tile ordering to balance per-queue descriptor totals: the
    # kernel is emission-rate-bound per Q7 core pair, so the slowest queue
    # sets the span. Position i sends half-0 to queue 2i%4 and half-1 to
    # queue (2i+1)%4; pick the remaining tile minimizing the running max.
    loads = [0.0] * USE_QUEUES
    remaining = set(range(n_tiles))
    tile_order = []
    ctr_sim = 0
    while remaining:
        qa = ctr_sim % USE_QUEUES
        qb = (ctr_sim + 1) % USE_QUEUES
        best, best_val = None, None
        for cand in remaining:
            l0 = float(ch_slot[cand, 0, :].sum())
            l1 = float(ch_slot[cand, 1, :].sum()) if ng == 2 else 0.0
            trial = loads.copy()
            trial[qa] += l0
            trial[qb] += l1
            val = (max(trial), -(l0 + l1))
            if best_val is None or val < best_val:
                best, best_val = cand, val
        tile_order.append(best)
        remaining.discard(best)
        l0 = float(ch_slot[best, 0, :].sum())
        l1 = float(ch_slot[best, 1, :].sum()) if ng == 2 else 0.0
        loads[qa] += l0
        if l0 > 0:
            ctr_sim += 1
        loads[qb if l0 > 0 else qa] += l1
        if l1 > 0:
            ctr_sim += 1
    gather_ctr = 0
    if True:
            for t in tile_order:
                xf_sb = fpool.tile([D, P], dt.float32, tag="xf")
                nc.sync.dma_start(
                    out=xf_sb[:], in_=xf[:, t * P:(t + 1) * P])

                # per-half gather (covers both dirs; per-dir split measured
                # WORSE: doubles per-gather fixed costs) + one-hot build
                xg_hd = {}
                m_h = {}
                for h in range(ng):
                    chs = int(ch_slot[t, h, 0] + ch_slot[t, h, 1])
                    if chs == 0:
                        continue
                    io = int(idx_off[t, h, 0])
                    xg = xgpool.tile([P, chs, xw], mdt, tag="xg")
                    if "no_gather" not in ABLATE:
                        nc.gpsimd.dma_gather(
                            out_ap=xg[:],
                            in_ap=xt[h * HALF:(h + 1) * HALF, :],
                            idxs_ap=idx_all[:, io:io + chs * 8],
                            num_idxs=chs * P,
                            num_idxs_reg=chs * P,
                            elem_size=xw,
                            single_packet=SINGLE_PACKET,
                            queue_num=gather_ctr % USE_QUEUES,
                        )
                        gather_ctr += 1
                    xg_hd[h] = xg
                    if "no_compute" in ABLATE:
                        continue
                    do = int(dstv_off[t, h, 0])
                    m_sb = mpool.tile([P, chs, P], mdt, tag="m")
                    # all operands get a unit-stride 2-element last dim so
                    # the DVE runs is_equal in 2x_1P mode: dstv is stored as
                    # duplicated pairs, iota/out are viewed as [.., 64, 2]
                    nc.vector.tensor_tensor(
                        out=m_sb[:].rearrange(
                            "p c (j two) -> p c j two", two=2),
                        in0=dstv_all[:, 2 * do:2 * (do + chs)]
                        .rearrange("p (c two) -> p c two", two=2)
                        [:, :, None, :].to_broadcast([P, chs, P // 2, 2]),
                        in1=iota_sb[:, :chs, :].rearrange(
                            "p c (j two) -> p c j two", two=2),
                        op=mybir.AluOpType.is_equal,
                    )
                    m_h[h] = m_sb

                if "no_compute" in ABLATE:
                    continue
                # agg accumulated TRANSPOSED: aggT[feat, dst] = sum_chunks
                # xg[e, feat]^T @ onehot[e, dst] -> the MLP consumes hT
                # directly, no PE transpose needed.
                r1_tiles = []
                for d in (0, 1):
                    agg_ps = aggpool.tile([P, P], dt.float32, tag="agg")
                    # chunk list: (half, local chunk in xg, chunk in m)
                    chunks = []
                    for h in range(ng):
                        base = 0 if d == 0 else int(ch_slot[t, h, 0])
                        for c in range(int(ch_slot[t, h, d])):
                            chunks.append((h, c, base + c))
                    for i, (h, c, mc) in enumerate(chunks):
                        nc.tensor.matmul(
                            out=agg_ps[:],
                            lhsT=xg_hd[h][:, mc, :],
                            rhs=m_h[h][:, mc, :],
                            start=(i == 0),
                            stop=(i == len(chunks) - 1),
                        )
                    ht_sb = fpool.tile([D, P], dt.float32, tag="ht")
                    if not chunks:
                        nc.vector.tensor_copy(out=ht_sb[:], in_=xf_sb[:])
                    else:
                        nc.vector.tensor_tensor(
                            out=ht_sb[:], in0=xf_sb[:], in1=agg_ps[:],
                            op=mybir.AluOpType.add)
                    l1_ps = mlppool.tile([P, D], dt.float32, tag="mlp")
                    nc.tensor.matmul(
                        out=l1_ps[:], lhsT=w1t_sb[:], rhs=ht_sb[:],
                        start=True, stop=True)
                    r1_sb = fpool.tile([P, D], dt.float32, tag="r1")
                    nc.scalar.activation(
                        out=r1_sb[:], in_=l1_ps[:],
                        func=mybir.ActivationFunctionType.Relu,
                        bias=b1_sb[:], scale=1.0)
                    r1_tiles.append(r1_sb)

                l2_ps = mlppool.tile([P, D], dt.float32, tag="mlp")
                nc.tensor.matmul(
                    out=l2_ps[:], lhsT=w2t_sb[:], rhs=r1_tiles[0][:],
                    start=True, stop=False)
                nc.tensor.matmul(
                    out=l2_ps[:], lhsT=w2t_sb[:], rhs=r1_tiles[1][:],
                    start=False, stop=True)
                out_sb = fpool.tile([P, D], dt.float32, tag="out")
                nc.scalar.activation(
                    out=out_sb[:], in_=l2_ps[:],
                    func=mybir.ActivationFunctionType.Relu,
                    bias=b2_sb[:], scale=0.5)
                nc.sync.dma_start(
                    out=y[:, t * P:(t + 1) * P], in_=out_sb[:])


_CACHE = {}
_LAST = {}


def _get_program(ch_slot, idx_off, dstv_off, toti, totd, ch_max):
    key = (tuple(ch_slot.ravel()), TILES_PER_CORE)
    if key not in _CACHE:
        _CACHE[key] = _build_program(
            ch_slot, idx_off, dstv_off, toti, totd, ch_max, TILES_PER_CORE)
    return _CACHE[key]


def kernel(x, edge_index, reverse_edge_index, w1, b1, w2, b2):
    from concourse.bass_utils import run_bass_kernel_spmd

    (ch_slot, idx_off, dstv_off, toti, totd, idx_cores, dstv_cores,
     xt, iota, ch_max, xft_cores) = _host_prep(
        x, edge_index, reverse_edge_index)
    nc = _get_program(ch_slot, idx_off, dstv_off, toti, totd, ch_max)

    w1t = np.ascontiguousarray(np.asarray(w1, np.float32).T)
    w2t = np.ascontiguousarray(np.asarray(w2, np.float32).T)
    b1c = np.ascontiguousarray(np.asarray(b1, np.float32)[:, None])
    b2c = np.ascontiguousarray(np.asarray(b2, np.float32)[:, None])

    in_maps = []
    for k in range(N_CORES):
        in_maps.append({
            "xt": xt,
            "idx": idx_cores[k],
            "dstv": dstv_cores[k],
            "iotar": iota,
            "w1t": w1t,
            "w2t": w2t,
            "b1c": b1c,
            "b2c": b2c,
            "xf": np.ascontiguousarray(xft_cores[k]),
        })

    res = run_bass_kernel_spmd(nc, in_maps, list(range(N_CORES)))
    _LAST["res"] = res
    y = np.concatenate([res.results[k]["y"] for k in range(N_CORES)], axis=1)
    return np.ascontiguousarray(y.T[:N_NODES])



# revision 13
# speedup vs baseline: 7.9371x; 7.9371x over previous
"""Bidirectional GINConv on 8 Trainium2 NeuronCores.

Strategy (dst-node sharding, zero collectives):
  - Pad node space to 50176 = 8 * 49 * 128; core k owns the 49 dst tiles
    (128 nodes each) of range [k*6272, (k+1)*6272).
  - Host groups edges by (dst tile, src half) with BOTH directions merged
    in one bucket (dir 0 edges first, then dir 1), padded to a per-slot
    chunk count (max over the 8 cores, so the SPMD program is
    shape-uniform); src indices become int16-local offsets.
  - Device: per (tile, half) one `dma_gather` fetches x rows (fp16,
    256B/row); a DVE is_equal against a two-bank iota builds one-hot dst
    matrices (dir 0 dstv values are 0..127 matched against bank 0, dir 1
    values 128..255 against bank 1, so a merged chunk feeds the right
    per-direction PSUM; only chunks straddling the dir boundary are
    matmul'd twice); the PE accumulates aggT = sum(xg^T @ onehot).
  - agg -> h, then the 2-layer MLP per direction, directions summed in
    PSUM, final relu((a+b)/2 + b2) on ACT, store.
  - Host concatenates the per-core [128, 6272] outputs and transposes.

Perf notes (measured on HW):
  - The kernel is Q7-emission-bound: dma_gather ucode costs ~8ns/idx per
    queue-pair, ~2.1ns/idx aggregate over 4 queues. 1 queue measured 3.6x
    worse; indirect_dma_start (qPoolDynamic HWDGE) measured ~20ns/row --
    both dead ends. So minimize idx slots: merging dirs per bucket cuts
    the 128-roundup padding from 12.9% to ~6%.
  - dynamic_dma_scratch_size=65536 -> 1024-desc rings/queue (~7 gathers
    deep): emission never stalls in await_space (32KB measured +5% worse).
  - idx table is loaded in blocks sized to the greedy tile order so the
    first gather starts ~2us in instead of waiting ~14us for one big DMA.
"""

import sys

import numpy as np

sys.path.insert(0, "/opt/trn_rl_repo")

P = 128
D = 128
N_NODES = 50000
N_EDGES = 800000
N_CORES = 8
TILES_PER_CORE = 49
NODES_PER_CORE = TILES_PER_CORE * P      # 6272
TABLE_ROWS = N_CORES * NODES_PER_CORE    # 50176
HALF = TABLE_ROWS // 2                   # 25088
NG = 2                                   # src halves (int16 idx range)

# round-robin SWDGE queues; 4 = all 8 Q7 cores (pairs)
N_SWDGE_QUEUES = 4
USE_QUEUES = 4
# sort each bucket's edges by src id -> ascending HBM addresses per DMA
# engine stream (row-buffer locality)
SORT_SRC = True
# idx load blocks (in greedy-tile-order positions): first block small so
# gather 0 starts immediately
IDX_BLOCKS = (2, 6, 12, TILES_PER_CORE)


def _tile_order_queues(ch_slot):
    """Greedy tile ordering balancing per-queue descriptor totals.

    Position i sends half-0 to queue 2i%4 and half-1 to queue (2i+1)%4;
    pick the remaining tile minimizing the running max queue load.
    """
    loads = [0.0] * USE_QUEUES
    remaining = set(range(TILES_PER_CORE))
    tile_order = []
    ctr = 0
    while remaining:
        qa = ctr % USE_QUEUES
        qb = (ctr + 1) % USE_QUEUES
        best, best_val = None, None
        for cand in remaining:
            l0 = float(ch_slot[cand, 0])
            l1 = float(ch_slot[cand, 1])
            trial = loads.copy()
            trial[qa] += l0
            trial[qb] += l1
            val = (max(trial), -(l0 + l1))
            if best_val is None or val < best_val:
                best, best_val = cand, val
        tile_order.append(best)
        remaining.discard(best)
        l0 = float(ch_slot[best, 0])
        l1 = float(ch_slot[best, 1])
        loads[qa] += l0
        if l0 > 0:
            ctr += 1
        loads[qb if l0 > 0 else qa] += l1
        if l1 > 0:
            ctr += 1
    return tile_order


def _host_prep(x, edge_index, reverse_edge_index):
    """Build per-core device input arrays (dir-merged buckets)."""
    n_buckets = N_CORES * TILES_PER_CORE * NG

    s = np.concatenate([np.asarray(edge_index[0], np.int64),
                        np.asarray(reverse_edge_index[0], np.int64)])
    t = np.concatenate([np.asarray(edge_index[1], np.int64),
                        np.asarray(reverse_edge_index[1], np.int64)])
    dirv = np.zeros(2 * N_EDGES, np.int64)
    dirv[N_EDGES:] = 1

    tile_id = t >> 7
    grp = (s >= HALF).astype(np.int64)
    key = tile_id * NG + grp
    if SORT_SRC:
        order = np.lexsort((s, dirv, key))
    else:
        order = np.lexsort((dirv, key))
    s_s = s[order]
    dl_s = (t[order] & 127) + 128 * dirv[order]  # dir 1 -> bank-1 values
    key_s = key[order]
    counts = np.bincount(key_s, minlength=n_buckets)
    c0 = np.bincount(key_s[dirv[order] == 0], minlength=n_buckets)

    cc = counts.reshape(N_CORES, TILES_PER_CORE, NG)
    cc0 = c0.reshape(N_CORES, TILES_PER_CORE, NG)
    # uniform chunk count per (tile, half): max over cores
    ch_slot = -(-cc.max(axis=0) // P)                      # [TILES, NG]
    # dir-boundary chunk range (compile-time, covering all cores)
    cb_lo = cc0.min(axis=0) // P                           # [TILES, NG]
    cb_hi = -(-cc0.max(axis=0) // P)                       # [TILES, NG]
    cb_hi = np.minimum(cb_hi, ch_slot)
    cb_lo = np.minimum(cb_lo, cb_hi)

    tile_order = _tile_order_queues(ch_slot)

    # column offsets laid out in tile_order so idx blocks are contiguous
    idx_off = np.zeros((TILES_PER_CORE, NG), np.int64)
    dstv_off = np.zeros((TILES_PER_CORE, NG), np.int64)
    acc = 0
    for tl in tile_order:
        for g in range(NG):
            idx_off[tl, g] = acc * 8
            dstv_off[tl, g] = acc
            acc += int(ch_slot[tl, g])
    toti = acc * 8
    totd = acc

    idx_cores = np.zeros((N_CORES, P, toti), np.int16)
    dstv_cores = np.full((N_CORES, P, 2 * totd), -1.0, np.float16)

    offs = np.zeros(n_buckets + 1, dtype=np.int64)
    np.cumsum(counts, out=offs[1:])
    for b in range(n_buckets):
        n = int(counts[b])
        tile, g = divmod(b, NG)
        core, tl = divmod(tile, TILES_PER_CORE)
        cap = int(ch_slot[tl, g]) * P
        if cap == 0:
            continue
        o = offs[b]
        io = int(idx_off[tl, g])
        src_l = np.zeros(cap, np.int16)
        src_l[:n] = (s_s[o:o + n] - g * HALF).astype(np.int16)
        # slot i -> [i % 16, i // 16], replicated to 8 groups of 16
        iw = src_l.reshape(cap // 16, 16).T
        idx_cores[core, :, io:io + cap // 16] = np.tile(iw, (8, 1))
        dl = np.full(cap, -1.0, np.float16)
        dl[:n] = dl_s[o:o + n].astype(np.float32).astype(np.float16)
        # dstv: slot i -> [i % 128, i // 128], values duplicated in pairs
        # for the DVE 2x_1P is_equal
        do = int(dstv_off[tl, g])
        dw = dl.reshape(cap // P, P).T
        dstv_cores[core, :, 2 * do:2 * (do + cap // P)] = \
            np.repeat(dw, 2, axis=1)

    x = np.asarray(x, np.float32)
    xt = np.zeros((TABLE_ROWS, D), np.float16)
    xt[:N_NODES] = x.astype(np.float16)

    ch_max = int(ch_slot.max())
    # two-bank iota: [P, 2, P] value at [.., b, j] = b*128 + j
    # (broadcast along the chunk dim on-device)
    iota = np.tile(np.arange(2 * P, dtype=np.float32).reshape(2, P),
                   (P, 1, 1)).astype(np.float16)

    # exact f32 x, sharded by core, transposed to [D, nodes]
    xf = np.zeros((TABLE_ROWS, D), np.float32)
    xf[:N_NODES] = x
    xf_cores = xf.reshape(N_CORES, NODES_PER_CORE, D)
    xft_cores = np.ascontiguousarray(xf_cores.transpose(0, 2, 1))
    return (ch_slot, cb_lo, cb_hi, idx_off, dstv_off, toti, totd,
            idx_cores, dstv_cores, xt, iota, ch_max, xft_cores, tile_order)


def _build_program(ch_slot, cb_lo, cb_hi, idx_off, dstv_off, toti, totd,
                   ch_max, tile_order):
    from concourse import bacc, mybir
    import concourse.tile as tile

    dt = mybir.dt
    nc = bacc.Bacc(
        "TRN2",
        target_bir_lowering=False,
        debug=False,
        enable_asserts=False,
        num_devices=1,
        # scratch/16/4queues = per-queue ring descs: 64KB -> 1024-desc
        # rings (~7 gathers deep) so emission never stalls in await_space
        dynamic_dma_scratch_size=65536,
        num_swdge_queues=N_SWDGE_QUEUES,
    )

    xt = nc.dram_tensor(
        "xt", [TABLE_ROWS, D], dt.float16, kind="ExternalInput").ap()
    idx = nc.dram_tensor(
        "idx", [P, toti], dt.int16, kind="ExternalInput").ap()
    dstv = nc.dram_tensor(
        "dstv", [P, 2 * totd], dt.float16, kind="ExternalInput").ap()
    iotar = nc.dram_tensor(
        "iotar", [P, 2, P], dt.float16, kind="ExternalInput").ap()
    w1t = nc.dram_tensor(
        "w1t", [D, D], dt.float32, kind="ExternalInput").ap()
    w2t = nc.dram_tensor(
        "w2t", [D, D], dt.float32, kind="ExternalInput").ap()
    b1c = nc.dram_tensor(
        "b1c", [D, 1], dt.float32, kind="ExternalInput").ap()
    b2c = nc.dram_tensor(
        "b2c", [D, 1], dt.float32, kind="ExternalInput").ap()
    xf = nc.dram_tensor(
        "xf", [D, NODES_PER_CORE], dt.float32, kind="ExternalInput").ap()
    y = nc.dram_tensor(
        "y", [D, TILES_PER_CORE * P], dt.float32, kind="ExternalOutput").ap()

    # idx-block column boundaries (tile_order positions -> columns)
    blk_cols = []
    prev = 0
    for stop in IDX_BLOCKS:
        tls = tile_order[prev:stop]
        w = int(sum(ch_slot[tl, g] for tl in tls for g in range(NG))) * 8
        blk_cols.append(w)
        prev = stop
    assert sum(blk_cols) == toti

    with tile.TileContext(nc) as tc:
        with (
            tc.tile_pool(name="const", bufs=1) as cpool,
            tc.tile_pool(name="xgp", bufs=9) as xgpool,
            tc.tile_pool(name="mp", bufs=8) as mpool,
            tc.tile_pool(name="fp", bufs=12) as fpool,
            tc.tile_pool(name="aggps", bufs=4, space="PSUM") as aggpool,
            tc.tile_pool(name="mlpps", bufs=4, space="PSUM") as mlppool,
        ):
            # idx loaded in blocks: gathers of block b wait only on their
            # block's DMA, so the pipeline starts ~2us in
            idx_blks = []
            col = 0
            for w in blk_cols:
                t_idx = cpool.tile([P, w], dt.int16)
                nc.sync.dma_start(out=t_idx[:], in_=idx[:, col:col + w])
                idx_blks.append((col, t_idx))
                col += w
            dstv_all = cpool.tile([P, 2 * totd], dt.float16)
            nc.sync.dma_start(out=dstv_all[:], in_=dstv[:])
            iota_sb = cpool.tile([P, 2, P], dt.float16)
            nc.sync.dma_start(out=iota_sb[:], in_=iotar[:])
            w1t_sb = cpool.tile([D, D], dt.float32)
            nc.sync.dma_start(out=w1t_sb[:], in_=w1t[:])
            w2t_sb = cpool.tile([D, D], dt.float32)
            nc.sync.dma_start(out=w2t_sb[:], in_=w2t[:])
            b1_sb = cpool.tile([D, 1], dt.float32)
            nc.sync.dma_start(out=b1_sb[:], in_=b1c[:])
            b2_sb = cpool.tile([D, 1], dt.float32)
            nc.sync.dma_start(out=b2_sb[:], in_=b2c[:])

            def idx_view(io, w):
                for col, t_idx in idx_blks:
                    if col <= io and io + w <= col + t_idx.shape[1]:
                        return t_idx[:, io - col:io - col + w]
                raise AssertionError("idx slice spans blocks")

            _build_tiles(
                nc, tc, mybir, dt, ch_slot, cb_lo, cb_hi, idx_off, dstv_off,
                idx_view, dstv_all, xf, y, xt, iota_sb, w1t_sb, w2t_sb,
                b1_sb, b2_sb, xgpool, mpool, fpool, aggpool, mlppool,
                tile_order)

    nc.compile()
    return nc


def _build_tiles(nc, tc, mybir, dt, ch_slot, cb_lo, cb_hi, idx_off,
                 dstv_off, idx_view, dstv_all, xf, y, xt, iota_sb, w1t_sb,
                 w2t_sb, b1_sb, b2_sb, xgpool, mpool, fpool, aggpool,
                 mlppool, tile_order):
    gather_ctr = 0
    m_max = int((ch_slot + (cb_hi - cb_lo)).max())
    for pos, t in enumerate(tile_order):
        xf_sb = fpool.tile([D, P], dt.float32, tag="xf")
        nc.sync.dma_start(out=xf_sb[:], in_=xf[:, t * P:(t + 1) * P])

        # per-half gather (one per (tile, half), both dirs merged)
        xg_h = {}
        m_h = {}
        for g in range(NG):
            chs = int(ch_slot[t, g])
            if chs == 0:
                continue
            lo, hi = int(cb_lo[t, g]), int(cb_hi[t, g])
            io = int(idx_off[t, g])
            xg = xgpool.tile([P, chs, D], dt.float16, tag="xg")
            nc.gpsimd.dma_gather(
                out_ap=xg[:],
                in_ap=xt[g * HALF:(g + 1) * HALF, :],
                idxs_ap=idx_view(io, chs * 8),
                num_idxs=chs * P,
                num_idxs_reg=chs * P,
                elem_size=D,
                single_packet=False,
                queue_num=gather_ctr % USE_QUEUES,
            )
            gather_ctr += 1
            xg_h[g] = xg

            # one-hot: bank-0 rows (chunks [0, hi)) for dir 0, bank-1 rows
            # (chunks [lo, chs)) for dir 1; m tile = [bank0 | bank1]
            do = int(dstv_off[t, g])
            mch = hi + (chs - lo)
            m_sb = mpool.tile([P, m_max, P], dt.float16, tag="m")
            for bank, c_a, c_b, m_ofs in ((0, 0, hi, 0),
                                          (1, lo, chs, hi - lo)):
                n = c_b - c_a
                if n <= 0:
                    continue
                nc.vector.tensor_tensor(
                    out=m_sb[:, c_a + m_ofs:c_b + m_ofs, :].rearrange(
                        "p c (j two) -> p c j two", two=2),
                    in0=dstv_all[:, 2 * (do + c_a):2 * (do + c_b)]
                    .rearrange("p (c two) -> p c two", two=2)
                    [:, :, None, :].to_broadcast([P, n, P // 2, 2]),
                    in1=iota_sb[:, bank, :].rearrange(
                        "p (j two) -> p j two", two=2)
                    [:, None, :, :].to_broadcast([P, n, P // 2, 2]),
                    op=mybir.AluOpType.is_equal,
                )
            m_h[g] = (m_sb, mch)

        # aggT[feat, dst] accumulated per dir; merged chunks straddling
        # the dir boundary are matmul'd once per bank
        r1_tiles = []
        for d in (0, 1):
            agg_ps = aggpool.tile([P, P], dt.float32, tag="agg")
            chunks = []  # (half, xg chunk, m chunk)
            for g in range(NG):
                chs = int(ch_slot[t, g])
                if chs == 0:
                    continue
                lo, hi = int(cb_lo[t, g]), int(cb_hi[t, g])
                if d == 0:
                    for c in range(hi):
                        chunks.append((g, c, c))
                else:
                    for c in range(lo, chs):
                        chunks.append((g, c, hi + c - lo))
            for i, (g, c, mc) in enumerate(chunks):
                nc.tensor.matmul(
                    out=agg_ps[:],
                    lhsT=xg_h[g][:, c, :],
                    rhs=m_h[g][0][:, mc, :],
                    start=(i == 0),
                    stop=(i == len(chunks) - 1),
                )
            ht_sb = fpool.tile([D, P], dt.float32, tag="ht")
            if not chunks:
                nc.vector.tensor_copy(out=ht_sb[:], in_=xf_sb[:])
            else:
                nc.vector.tensor_tensor(
                    out=ht_sb[:], in0=xf_sb[:], in1=agg_ps[:],
                    op=mybir.AluOpType.add)
            l1_ps = mlppool.tile([P, D], dt.float32, tag="mlp")
            nc.tensor.matmul(
                out=l1_ps[:], lhsT=w1t_sb[:], rhs=ht_sb[:],
                start=True, stop=True)
            r1_sb = fpool.tile([P, D], dt.float32, tag="r1")
            nc.scalar.activation(
                out=r1_sb[:], in_=l1_ps[:],
                func=mybir.ActivationFunctionType.Relu,
                bias=b1_sb[:], scale=1.0)
            r1_tiles.append(r1_sb)

        l2_ps = mlppool.tile([P, D], dt.float32, tag="mlp")
        nc.tensor.matmul(
            out=l2_ps[:], lhsT=w2t_sb[:], rhs=r1_tiles[0][:],
            start=True, stop=False)
        nc.tensor.matmul(
            out=l2_ps[:], lhsT=w2t_sb[:], rhs=r1_tiles[1][:],
            start=False, stop=True)
        out_sb = fpool.tile([P, D], dt.float32, tag="out")
        nc.scalar.activation(
            out=out_sb[:], in_=l2_ps[:],
            func=mybir.ActivationFunctionType.Relu,
            bias=b2_sb[:], scale=0.5)
        nc.sync.dma_start(out=y[:, t * P:(t + 1) * P], in_=out_sb[:])


_CACHE = {}
_LAST = {}


def _get_program(ch_slot, cb_lo, cb_hi, idx_off, dstv_off, toti, totd,
                 ch_max, tile_order):
    key = (tuple(ch_slot.ravel()), tuple(cb_lo.ravel()),
           tuple(cb_hi.ravel()))
    if key not in _CACHE:
        _CACHE[key] = _build_program(
            ch_slot, cb_lo, cb_hi, idx_off, dstv_off, toti, totd, ch_max,
            tile_order)
    return _CACHE[key]


def kernel(x, edge_index, reverse_edge_index, w1, b1, w2, b2):
    from concourse.bass_utils import run_bass_kernel_spmd

    (ch_slot, cb_lo, cb_hi, idx_off, dstv_off, toti, totd, idx_cores,
     dstv_cores, xt, iota, ch_max, xft_cores, tile_order) = _host_prep(
        x, edge_index, reverse_edge_index)
    nc = _get_program(ch_slot, cb_lo, cb_hi, idx_off, dstv_off, toti, totd,
                      ch_max, tile_order)

    w1t = np.ascontiguousarray(np.asarray(w1, np.float32).T)
    w2t = np.ascontiguousarray(np.asarray(w2, np.float32).T)
    b1c = np.ascontiguousarray(np.asarray(b1, np.float32)[:, None])
    b2c = np.ascontiguousarray(np.asarray(b2, np.float32)[:, None])

    in_maps = []
    for k in range(N_CORES):
        in_maps.append({
            "xt": xt,
            "idx": idx_cores[k],
            "dstv": dstv_cores[k],
            "iotar": iota,
            "w1t": w1t,
            "w2t": w2t,
            "b1c": b1c,
            "b2c": b2c,
            "xf": np.ascontiguousarray(xft_cores[k]),
        })

    res = run_bass_kernel_spmd(nc, in_maps, list(range(N_CORES)))
    _LAST["res"] = res
    y = np.concatenate([res.results[k]["y"] for k in range(N_CORES)], axis=1)
    return np.ascontiguousarray(y.T[:N_NODES])
